# revision 45
# baseline (speedup 1.0000x reference)
"""Causal multi-head attention (QKV projection + softmax(QK^T)V) on 8 TRN2 NeuronCores.

Problem: x[4,2048,1024] @ W_qkv[1024,3072] + b_qkv -> 16-head causal attention -> [4,2048,1024].

Sharding: core i = (batch bi=i//2, head-group hg=i%2). Each core handles 1 batch x 8 heads,
fully data/tensor-parallel (no collectives). Host pre-arranges per-core weight shards:
  - wqk [1024,1024] fp16: Q then K columns, head-PAIR-stacked (col chunk c of 128 = heads
    (2c,2c+1) x 64 dims) so QKV^T matmul output chunks are directly the [hd,n] stacked
    layout the attention stage wants, and K=64 attention matmuls can be row-tiled in pairs.
  - wv [1024,520] fp16: V columns with per-head stride 65; col 65h+64 is zero, and the
    matching bias entry is 1.0, so the "ones column" used for softmax denominators is
    produced by the same bias-row matmul that applies b_v.
Device pipeline per core:
  x^T via xbar DMA-transpose (fp16) -> QKV^T matmuls (Q^T,K^T in [hd,n], V natural)
  -> S^T = K Q^T (row-tiled pairs of heads) -> ScalarE Exp(scale=1/8) PSUM->SBUF = P^T fp16
  -> causal mask (multiply 128x128 diagonal blocks only) -> PV matmuls accumulate
  [q,64]+denominator -> reciprocal * scale epilogue -> DMA out [2048,512] f32.
Scheduling: ScalarE exp (~174us busy) is the critical engine; QKV matmul work is queued as
"filler" pulled into the attention loop between S^T groups, and each stripe's PV matmuls
are deferred into the next stripe's S^T/exp loop, so the PE always has work while ACT exps.
"""

import numpy as np

import concourse.bass as bass
import concourse.tile as tile
from concourse import bacc, mybir
from concourse import bass_utils

F16 = mybir.dt.float16
F32 = mybir.dt.float32

B, N, D = 4, 2048, 1024
H = 16  # global heads
HD = 64
HL = 8  # heads per core
N_CORES = 8
P = 128
NT = N // P  # 16 token tiles
KC = D // P  # 8 contraction chunks
VW = HL * (HD + 1)  # 520
VH = VW // 2  # 260

_cache = {}


def _build():
    nc = bacc.Bacc("TRN2", target_bir_lowering=False, debug=False)

    x_d = nc.dram_tensor("x", [D, N], F16, kind="ExternalInput").ap()  # x^T, host-transposed
    wqk_d = nc.dram_tensor("wqk", [D, 1024], F16, kind="ExternalInput").ap()
    wv_d = nc.dram_tensor("wv", [D, VW], F16, kind="ExternalInput").ap()
    bqk_d = nc.dram_tensor("bqk", [P, 8], F32, kind="ExternalInput").ap()
    bv_d = nc.dram_tensor("bv", [1, VW], F16, kind="ExternalInput").ap()
    tri_d = nc.dram_tensor("tri", [P, P], F16, kind="ExternalInput").ap()
    ones_d = nc.dram_tensor("ones1", [1, P], F16, kind="ExternalInput").ap()
    out_d = nc.dram_tensor("out", [N, HL * HD], F32, kind="ExternalOutput").ap()

    wqk_r = wqk_d.rearrange("(k p) n -> p k n", p=P)
    wv_r = wv_d.rearrange("(k p) n -> p k n", p=P)

    with tile.TileContext(nc) as tc:
        with (
            tc.tile_pool(name="const", bufs=1) as cpool,
            tc.tile_pool(name="pt", bufs=2) as ptpool,
            tc.tile_pool(name="opair", bufs=3) as oppool,
            tc.tile_pool(name="misc", bufs=6) as mpool,
            tc.tile_pool(name="ps_mm", bufs=2, space="PSUM") as ps_mm,
            tc.tile_pool(name="ps_s", bufs=2, space="PSUM") as ps_s,
            tc.tile_pool(name="ps_o", bufs=2, space="PSUM") as ps_o,
        ):
            # ---- constants / inputs to SBUF ----
            xt_sb = cpool.tile([P, KC, N], F16, name="xt_sb")  # x^T, 8 chunks of [128, 2048]
            wqk_sb = cpool.tile([P, KC, 1024], F16, name="wqk_sb")
            wv_sb = cpool.tile([P, KC, VW], F16, name="wv_sb")
            bqk_sb = cpool.tile([P, 8], F32, name="bqk_sb")
            bv_sb = cpool.tile([1, VW], F16, name="bv_sb")
            tri_sb = cpool.tile([P, P], F16, name="tri_sb")
            ones_sb = cpool.tile([1, P], F16, name="ones_sb")
            qt_sb = cpool.tile([P, 4, N], F16, name="qt_sb")  # Q^T pair-stacked
            # K^T zero-padded per head: head h_l occupies rows 64*(h_l%2).. , other half 0,
            # so S^T matmuls run K=128 (FWL) against the pair-stacked Q^T rhs.
            kt_sb = cpool.tile([P, HL, N], F16, name="kt_sb")
            v_sb = cpool.tile([P, NT, VW], F16, name="v_sb")

            nc.gpsimd.memset(kt_sb[:], 0.0)
            # Preload the exp table set (~2.7us) during the DMA fill, so the first
            # real softmax exp doesn't pay ACT_TABLE_LOAD.
            warm = mpool.tile([1, 8], F32, tag="warm", name="warm")
            nc.gpsimd.memset(warm[:], 0.0)
            nc.scalar.activation(warm[:], warm[:], mybir.ActivationFunctionType.Exp)
            # ALL bulk input DMA rides the sync HWDGE ring: the scalar ring would put
            # DMA-trigger instructions (which stall on ring credit) ahead of the first
            # EXPs in ScalarE's instruction stream, delaying softmax by ~20us.
            nc.sync.dma_start(bqk_sb[:], bqk_d)
            nc.sync.dma_start(tri_sb[:], tri_d)
            nc.sync.dma_start(bv_sb[:], bv_d)
            nc.sync.dma_start(ones_sb[:], ones_d)

            # x^T streamed stripe-major in [128,512] pieces: stripe-0 of all chunks
            # lands first so QK(.,0) -> S^T(0,0) -> exp starts ~10us in.
            def x_piece(k, tt):
                nc.sync.dma_start(
                    xt_sb[:, k, tt * 512 : (tt + 1) * 512],
                    x_d[k * P : (k + 1) * P, tt * 512 : (tt + 1) * 512],
                )

            # wqk is pair-major host-side (pair p: Q at 256p, K at 256p+128), so the
            # first S^T only waits on the 256-col pair-0 pieces.
            for k in range(KC):
                x_piece(k, 0)
                nc.sync.dma_start(wqk_sb[:, k, 0:256], wqk_r[:, k, 0:256])
            for k in range(KC):
                nc.sync.dma_start(wv_sb[:, k, :], wv_r[:, k, :])
            for k in range(KC):
                nc.sync.dma_start(wqk_sb[:, k, 256:1024], wqk_r[:, k, 256:1024])
            for k in range(KC):
                x_piece(k, 1)
            for k in range(KC):
                x_piece(k, 2)
            for k in range(KC):
                x_piece(k, 3)

            done_qk = set()
            done_v = set()

            def emit_qk(c, tt):
                """QKV^T matmul tile for col-chunk c, token stripe tt."""
                if (c, tt) in done_qk:
                    return
                done_qk.add((c, tt))
                pr = c % 4
                pq = ps_mm.tile([P, 512], F32, tag="mm", name=f"pq_{c}_{tt}")
                col0 = 256 * (c % 4) + (0 if c < 4 else 128)
                for k in range(KC):
                    nc.tensor.matmul(
                        pq[:],
                        lhsT=wqk_sb[:, k, col0 : col0 + P],
                        rhs=xt_sb[:, k, tt * 512 : (tt + 1) * 512],
                        start=(k == 0),
                        stop=(k == KC - 1),
                    )
                def badd(out, in_, b):
                    nc.vector.tensor_scalar_add(out, in_, b)

                if c < 4:
                    badd(
                        qt_sb[:, pr, tt * 512 : (tt + 1) * 512], pq[:], bqk_sb[:, c : c + 1]
                    )
                else:
                    for hh in (0, 1):
                        rows = slice(64 * hh, 64 * hh + 64)
                        badd(
                            kt_sb[rows, 2 * pr + hh, tt * 512 : (tt + 1) * 512],
                            pq[rows, :],
                            bqk_sb[rows, c : c + 1],
                        )

            def emit_v(j, half):
                """V (augmented) for token tile j, half (260 cols each)."""
                if (j, half) in done_v:
                    return
                done_v.add((j, half))
                pv = ps_mm.tile([P, VH], F32, tag="mm", name=f"pv_{j}_{half}")
                for k in range(KC):
                    nc.tensor.matmul(
                        pv[:],
                        lhsT=xt_sb[:, k, j * P : (j + 1) * P],
                        rhs=wv_sb[:, k, half * VH : (half + 1) * VH],
                        start=(k == 0),
                        stop=False,
                    )
                nc.tensor.matmul(
                    pv[:],
                    lhsT=ones_sb[0:1, :],
                    rhs=bv_sb[0:1, half * VH : (half + 1) * VH],
                    start=False,
                    stop=True,
                )
                nc.vector.tensor_copy(v_sb[:, j, half * VH : (half + 1) * VH], pv[:])

            # Filler queue: PE work pulled into the attention loop between S^T groups,
            # ordered to match t-major demand (QK stripes tt, then V tiles of stripe tt).
            filler = []
            for tt in range(4):
                for pr in range(4):
                    if (pr, tt) != (0, 0):
                        filler += [("qk", pr, tt), ("qk", pr + 4, tt)]
                filler += [("v", j, half) for j in range(4 * tt, 4 * tt + 4) for half in (0, 1)]
            state = {"i": 0}

            def pull(n):
                while n > 0 and state["i"] < len(filler):
                    it = filler[state["i"]]
                    state["i"] += 1
                    if it[0] == "v":
                        if (it[1], it[2]) in done_v:
                            continue
                        emit_v(it[1], it[2])
                    else:
                        if (it[1], it[2]) in done_qk:
                            continue
                        emit_qk(it[1], it[2])
                    n -= 1

            def emit_pv(p, t, pt, r):
                """PV + epilogue + out DMA for q-block i = 4t+r of pair p.
                Both heads' accumulators share one PSUM bank ([128, 2, 65])."""
                i = 4 * t + r
                opair = oppool.tile([P, P], F32, tag="op", name=f"op_{p}_{i}")
                po = ps_o.tile([P, 2, 65], F32, tag="o", name=f"po_{p}_{i}")
                for hh in (0, 1):
                    for j in range(i + 1):
                        nc.tensor.matmul(
                            po[:, hh, :],
                            lhsT=pt[:, hh, j, r * P : (r + 1) * P],
                            rhs=v_sb[:, j, 65 * (2 * p + hh) : 65 * (2 * p + hh) + 65],
                            start=(j == 0),
                            stop=(j == i),
                        )
                rc = mpool.tile([P, 2], F32, tag="rc", name=f"rc_{p}_{i}")
                nc.vector.reciprocal(rc[:], po[:, :, 64])
                for hh in (0, 1):
                    nc.vector.tensor_scalar_mul(
                        opair[:, 64 * hh : 64 * hh + 64], po[:, hh, 0:64], rc[:, hh : hh + 1]
                    )
                nc.sync.dma_start(out_d[i * P : (i + 1) * P, p * P : (p + 1) * P], opair[:])

            # Prologue: just the first QK stripes so S^T (0,0) can start ASAP.
            emit_qk(0, 0)
            emit_qk(4, 0)

            pv_queue = []
            blocks = [(pos, t, p) for pos, t in enumerate((0, 1, 3, 2)) for p in range(4)]
            for n, (pos, t, p) in enumerate(blocks):
                    for tt in range(t + 1):
                        emit_qk(p, tt)
                        emit_qk(4 + p, tt)
                    # QK tiles the NEXT block's S^T will need: emitted from this
                    # block's group hooks so the boundary has no QK burst.
                    nxt_qk = []
                    if n + 1 < len(blocks):
                        _, tn, pn = blocks[n + 1]
                        nxt_qk = [
                            (c, tt)
                            for tt in range(tn + 1)
                            for c in (pn, 4 + pn)
                            if (c, tt) not in done_qk
                        ]
                    # pt layout: [128, hh, chunk, 512]
                    pt = ptpool.tile([P, 2, 16, 512], F16, tag="pt", name=f"pt_{p}_{t}")
                    # V tiles this stripe's PV needs, spread across the group loop
                    vpend = [
                        (j, half)
                        for j in range(4 * t, 4 * t + 4)
                        for half in (0, 1)
                        if (j, half) not in done_v
                    ]

                    def group_hooks(pos=pos, vpend=vpend, nxt_qk=nxt_qk):
                        if pv_queue:
                            emit_pv(*pv_queue.pop(0))
                        if vpend:
                            emit_v(*vpend.pop(0))
                        if nxt_qk:
                            emit_qk(*nxt_qk.pop(0))
                            return
                        # pace filler: defer to the exp-heavy late rounds (prefix
                        # preconditions are met by the forced drains)
                        state["g"] = state.get("g", 0) + 1
                        if pos >= 2:
                            pull(1)

                    # S^T + exp in groups of 2 chunks per head; diagonal chunks only
                    # compute the causal-valid columns (stale psum prefix is bounded
                    # old scores: exp'd then never consumed).
                    for g in range(2 * t + 2):
                        psA = ps_s.tile([P, 2, 512], F32, tag="s", name=f"psA_{p}_{t}_{g}")
                        psB = ps_s.tile([P, 2, 512], F32, tag="s", name=f"psB_{p}_{t}_{g}")
                        for jj in (0, 1):
                            j = 2 * g + jj
                            q0 = 128 * (j - 4 * t) if j >= 4 * t else 0
                            for hh, ps in ((0, psA), (1, psB)):
                                nc.tensor.matmul(
                                    ps[:, jj, q0:512],
                                    lhsT=kt_sb[:, 2 * p + hh, j * P : (j + 1) * P],
                                    rhs=qt_sb[:, p, t * 512 + q0 : (t + 1) * 512],
                                    start=True,
                                    stop=True,
                                )
                        for hh, ps in ((0, psA), (1, psB)):
                            if g == 2 * t + 1:
                                # fully-diagonal group: exp only the causal-valid
                                # suffixes (contiguous slices)
                                nc.scalar.activation(
                                    pt[:, hh, 2 * g, 256:512],
                                    ps[:, 0, 256:512],
                                    mybir.ActivationFunctionType.Exp,
                                    scale=0.125,
                                )
                                nc.scalar.activation(
                                    pt[:, hh, 2 * g + 1, 384:512],
                                    ps[:, 1, 384:512],
                                    mybir.ActivationFunctionType.Exp,
                                    scale=0.125,
                                )
                            else:
                                nc.scalar.activation(
                                    pt[:, hh, 2 * g : 2 * g + 2, :],
                                    ps[:],
                                    mybir.ActivationFunctionType.Exp,
                                    scale=0.125,
                                )
                        group_hooks()
                    while pv_queue:
                        emit_pv(*pv_queue.pop(0))
                    # causal mask on diagonal 128x128 blocks (GpSimd: it's idle, and
                    # this keeps DVE free for psum drains/epilogues)
                    for hh in (0, 1):
                        for r in range(4):
                            j = 4 * t + r
                            blk = pt[:, hh, j, r * P : (r + 1) * P]
                            nc.gpsimd.tensor_mul(blk, blk, tri_sb[:])
                    # V tiles this stripe's PV will need (PV runs during next stripe)
                    for j in range(4 * t + 4):
                        emit_v(j, 0)
                        emit_v(j, 1)
                    pv_queue = [(p, t, pt, r) for r in range(4)]
            while pv_queue:
                emit_pv(*pv_queue.pop(0))
            pull(len(filler))  # safety: flush

    nc.compile()
    return nc


def get_nc():
    if "nc" not in _cache:
        _cache["nc"] = _build()
    return _cache["nc"]


def _prep_core_inputs(x, W, b, bi, hg):
    h0 = hg * HL
    Wq = W[:, 0:D].reshape(D, H, HD)
    Wk = W[:, D : 2 * D].reshape(D, H, HD)
    Wv = W[:, 2 * D :].reshape(D, H, HD)
    bq = b[0:D].reshape(H, HD)
    bk = b[D : 2 * D].reshape(H, HD)
    bv = b[2 * D :].reshape(H, HD)

    # pair-major: pair p occupies cols [256p, 256p+256) as [Q pair | K pair]
    wqk = np.empty((D, 1024), np.float32)
    bqk = np.empty((P, 8), np.float32)
    for c in range(4):
        for half in range(2):
            h = h0 + 2 * c + half
            sl = slice(256 * c + half * HD, 256 * c + half * HD + HD)
            wqk[:, sl] = Wq[:, h]
            bqk[half * HD : (half + 1) * HD, c] = bq[h]
            sl = slice(256 * c + P + half * HD, 256 * c + P + half * HD + HD)
            wqk[:, sl] = Wk[:, h]
            bqk[half * HD : (half + 1) * HD, 4 + c] = bk[h]

    wv_aug = np.zeros((D, VW), np.float32)
    bv_aug = np.zeros((VW,), np.float32)
    for hl in range(HL):
        wv_aug[:, 65 * hl : 65 * hl + HD] = Wv[:, h0 + hl]
        bv_aug[65 * hl : 65 * hl + HD] = bv[h0 + hl]
        bv_aug[65 * hl + HD] = 1.0

    tri = np.triu(np.ones((P, P), np.float32))  # tri[k, q] = 1 where q >= k

    return {
        "x": np.ascontiguousarray(x[bi].astype(np.float16).T),
        "wqk": wqk.astype(np.float16),
        "wv": wv_aug.astype(np.float16),
        "bqk": bqk,
        "bv": bv_aug[None, :].astype(np.float16),
        "tri": tri.astype(np.float16),
        "ones1": np.ones((1, P), np.float16),
    }


def make_in_maps(x, W_qkv, b_qkv):
    x = np.asarray(x, dtype=np.float32)
    W = np.asarray(W_qkv, dtype=np.float32)
    b = np.asarray(b_qkv, dtype=np.float32)
    return [_prep_core_inputs(x, W, b, i // 2, i % 2) for i in range(N_CORES)]


def assemble(results):
    out = np.empty((B, N, D), np.float32)
    for i in range(N_CORES):
        bi, hg = i // 2, i % 2
        out[bi, :, hg * 512 : (hg + 1) * 512] = results[i]["out"]
    return out


def run(x, W_qkv, b_qkv, trace=False, tmpdir=None):
    nc = get_nc()
    in_maps = make_in_maps(x, W_qkv, b_qkv)
    res = bass_utils.run_bass_kernel_spmd(
        nc, in_maps, core_ids=list(range(N_CORES)), trace=trace, tmpdir=tmpdir
    )
    return assemble(res.results), res


def kernel(x, W_qkv, b_qkv):
    out, _ = run(x, W_qkv, b_qkv)
    return out


# revision 46
# speedup vs baseline: 1.0723x; 1.0723x over previous
"""Causal multi-head attention (QKV projection + softmax(QK^T)V) on 8 TRN2 NeuronCores.

Problem: x[4,2048,1024] @ W_qkv[1024,3072] + b_qkv -> 16-head causal attention -> [4,2048,1024].

Sharding: core i = (batch bi=i//2, head-group hg=i%2). Each core handles 1 batch x 8 heads,
fully data/tensor-parallel (no collectives). Host pre-arranges per-core weight shards:
  - wqk [1024,1024] fp16: Q then K columns, head-PAIR-stacked (col chunk c of 128 = heads
    (2c,2c+1) x 64 dims) so QKV^T matmul output chunks are directly the [hd,n] stacked
    layout the attention stage wants, and K=64 attention matmuls can be row-tiled in pairs.
  - wv [1024,520] fp16: V columns with per-head stride 65; col 65h+64 is zero, and the
    matching bias entry is 1.0, so the "ones column" used for softmax denominators is
    produced by the same bias-row matmul that applies b_v.
Device pipeline per core:
  x^T via xbar DMA-transpose (fp16) -> QKV^T matmuls (Q^T,K^T in [hd,n], V natural)
  -> S^T = K Q^T (row-tiled pairs of heads) -> ScalarE Exp(scale=1/8) PSUM->SBUF = P^T fp16
  -> causal mask (multiply 128x128 diagonal blocks only) -> PV matmuls accumulate
  [q,64]+denominator -> reciprocal * scale epilogue -> DMA out [2048,512] f32.
Scheduling: ScalarE exp (~174us busy) is the critical engine; QKV matmul work is queued as
"filler" pulled into the attention loop between S^T groups, and each stripe's PV matmuls
are deferred into the next stripe's S^T/exp loop, so the PE always has work while ACT exps.
"""

import numpy as np

import concourse.bass as bass
import concourse.tile as tile
from concourse import bacc, mybir
from concourse import bass_utils

F16 = mybir.dt.float16
F32 = mybir.dt.float32

B, N, D = 4, 2048, 1024
H = 16  # global heads
HD = 64
HL = 8  # heads per core
N_CORES = 8
P = 128
NT = N // P  # 16 token tiles
KC = D // P  # 8 contraction chunks
VW = HL * (HD + 1)  # 520
VH = VW // 2  # 260

_cache = {}


def _build():
    nc = bacc.Bacc("TRN2", target_bir_lowering=False, debug=False)

    x_d = nc.dram_tensor("x", [D, N], F16, kind="ExternalInput").ap()  # x^T, host-transposed
    wqk_d = nc.dram_tensor("wqk", [D, 1024], F16, kind="ExternalInput").ap()
    wv_d = nc.dram_tensor("wv", [D, VW], F16, kind="ExternalInput").ap()
    bqk_d = nc.dram_tensor("bqk", [P, 8], F32, kind="ExternalInput").ap()
    bv_d = nc.dram_tensor("bv", [1, VW], F16, kind="ExternalInput").ap()
    tri_d = nc.dram_tensor("tri", [P, P], F16, kind="ExternalInput").ap()
    ones_d = nc.dram_tensor("ones1", [1, P], F16, kind="ExternalInput").ap()
    out_d = nc.dram_tensor("out", [N, HL * HD], F32, kind="ExternalOutput").ap()

    wqk_r = wqk_d.rearrange("(k p) n -> p k n", p=P)
    wv_r = wv_d.rearrange("(k p) n -> p k n", p=P)

    with tile.TileContext(nc) as tc:
        with (
            tc.tile_pool(name="const", bufs=1) as cpool,
            tc.tile_pool(name="pt", bufs=2) as ptpool,
            tc.tile_pool(name="opair", bufs=3) as oppool,
            tc.tile_pool(name="misc", bufs=6) as mpool,
            tc.tile_pool(name="ps_mm", bufs=2, space="PSUM") as ps_mm,
            tc.tile_pool(name="ps_s", bufs=2, space="PSUM") as ps_s,
            tc.tile_pool(name="ps_o", bufs=2, space="PSUM") as ps_o,
        ):
            # ---- constants / inputs to SBUF ----
            xt_sb = cpool.tile([P, KC, N], F16, name="xt_sb")  # x^T, 8 chunks of [128, 2048]
            wqk_sb = cpool.tile([P, KC, 1024], F16, name="wqk_sb")
            wv_sb = cpool.tile([P, KC, VW], F16, name="wv_sb")
            bqk_sb = cpool.tile([P, 8], F32, name="bqk_sb")
            bv_sb = cpool.tile([1, VW], F16, name="bv_sb")
            tri_sb = cpool.tile([P, P], F16, name="tri_sb")
            ones_sb = cpool.tile([1, P], F16, name="ones_sb")
            qt_sb = cpool.tile([P, 4, N], F16, name="qt_sb")  # Q^T pair-stacked
            # K^T zero-padded per head: head h_l occupies rows 64*(h_l%2).. , other half 0,
            # so S^T matmuls run K=128 (FWL) against the pair-stacked Q^T rhs.
            kt_sb = cpool.tile([P, HL, N], F16, name="kt_sb")
            v_sb = cpool.tile([P, NT, VW], F16, name="v_sb")

            nc.gpsimd.memset(kt_sb[:], 0.0)
            # Preload the exp table set (~2.7us) during the DMA fill, so the first
            # real softmax exp doesn't pay ACT_TABLE_LOAD.
            warm = mpool.tile([1, 8], F32, tag="warm", name="warm")
            nc.gpsimd.memset(warm[:], 0.0)
            nc.scalar.activation(warm[:], warm[:], mybir.ActivationFunctionType.Exp)
            # ALL bulk input DMA rides the sync HWDGE ring: the scalar ring would put
            # DMA-trigger instructions (which stall on ring credit) ahead of the first
            # EXPs in ScalarE's instruction stream, delaying softmax by ~20us.
            nc.sync.dma_start(bqk_sb[:], bqk_d)
            nc.sync.dma_start(tri_sb[:], tri_d)
            nc.sync.dma_start(bv_sb[:], bv_d)
            nc.sync.dma_start(ones_sb[:], ones_d)

            # x^T streamed stripe-major in [128,512] pieces: stripe-0 of all chunks
            # lands first so QK(.,0) -> S^T(0,0) -> exp starts ~10us in.
            def x_piece(k, tt):
                nc.sync.dma_start(
                    xt_sb[:, k, tt * 512 : (tt + 1) * 512],
                    x_d[k * P : (k + 1) * P, tt * 512 : (tt + 1) * 512],
                )

            # wqk is pair-major host-side (pair p: Q at 256p, K at 256p+128), so the
            # first S^T only waits on the 256-col pair-0 pieces.
            for k in range(KC):
                x_piece(k, 0)
                nc.sync.dma_start(wqk_sb[:, k, 0:256], wqk_r[:, k, 0:256])
            for k in range(KC):
                nc.sync.dma_start(wv_sb[:, k, :], wv_r[:, k, :])
            for k in range(KC):
                nc.sync.dma_start(wqk_sb[:, k, 256:1024], wqk_r[:, k, 256:1024])
            for k in range(KC):
                x_piece(k, 1)
            for k in range(KC):
                x_piece(k, 2)
            for k in range(KC):
                x_piece(k, 3)

            done_qk = set()
            done_v = set()

            def emit_qk(c, tt):
                """QKV^T matmul tile for col-chunk c, token stripe tt."""
                if (c, tt) in done_qk:
                    return
                done_qk.add((c, tt))
                pr = c % 4
                pq = ps_mm.tile([P, 512], F32, tag="mm", name=f"pq_{c}_{tt}")
                col0 = 256 * (c % 4) + (0 if c < 4 else 128)
                for k in range(KC):
                    nc.tensor.matmul(
                        pq[:],
                        lhsT=wqk_sb[:, k, col0 : col0 + P],
                        rhs=xt_sb[:, k, tt * 512 : (tt + 1) * 512],
                        start=(k == 0),
                        stop=(k == KC - 1),
                    )
                def badd(out, in_, b):
                    nc.vector.tensor_scalar_add(out, in_, b)

                if c < 4:
                    badd(
                        qt_sb[:, pr, tt * 512 : (tt + 1) * 512], pq[:], bqk_sb[:, c : c + 1]
                    )
                else:
                    for hh in (0, 1):
                        rows = slice(64 * hh, 64 * hh + 64)
                        badd(
                            kt_sb[rows, 2 * pr + hh, tt * 512 : (tt + 1) * 512],
                            pq[rows, :],
                            bqk_sb[rows, c : c + 1],
                        )

            def emit_v(j, half):
                """V (augmented) for token tile j, half (260 cols each)."""
                if (j, half) in done_v:
                    return
                done_v.add((j, half))
                pv = ps_mm.tile([P, VH], F32, tag="mm", name=f"pv_{j}_{half}")
                for k in range(KC):
                    nc.tensor.matmul(
                        pv[:],
                        lhsT=xt_sb[:, k, j * P : (j + 1) * P],
                        rhs=wv_sb[:, k, half * VH : (half + 1) * VH],
                        start=(k == 0),
                        stop=False,
                    )
                nc.tensor.matmul(
                    pv[:],
                    lhsT=ones_sb[0:1, :],
                    rhs=bv_sb[0:1, half * VH : (half + 1) * VH],
                    start=False,
                    stop=True,
                )
                nc.vector.tensor_copy(v_sb[:, j, half * VH : (half + 1) * VH], pv[:])

            # Filler queue: PE work pulled into the attention loop between S^T groups,
            # ordered to match t-major demand (QK stripes tt, then V tiles of stripe tt).
            filler = []
            for tt in range(4):
                for pr in range(4):
                    if (pr, tt) != (0, 0):
                        filler += [("qk", pr, tt), ("qk", pr + 4, tt)]
                filler += [("v", j, half) for j in range(4 * tt, 4 * tt + 4) for half in (0, 1)]
            state = {"i": 0}

            def pull(n):
                while n > 0 and state["i"] < len(filler):
                    it = filler[state["i"]]
                    state["i"] += 1
                    if it[0] == "v":
                        if (it[1], it[2]) in done_v:
                            continue
                        emit_v(it[1], it[2])
                    else:
                        if (it[1], it[2]) in done_qk:
                            continue
                        emit_qk(it[1], it[2])
                    n -= 1

            def emit_pv(p, t, pt, r):
                """PV + epilogue + out DMA for q-block i = 4t+r of pair p.
                Both heads' accumulators share one PSUM bank ([128, 2, 65])."""
                i = 4 * t + r
                opair = oppool.tile([P, P], F32, tag="op", name=f"op_{p}_{i}")
                po = ps_o.tile([P, 2, 65], F32, tag="o", name=f"po_{p}_{i}")
                for hh in (0, 1):
                    for j in range(i + 1):
                        nc.tensor.matmul(
                            po[:, hh, :],
                            lhsT=pt[:, hh, j, r * P : (r + 1) * P],
                            rhs=v_sb[:, j, 65 * (2 * p + hh) : 65 * (2 * p + hh) + 65],
                            start=(j == 0),
                            stop=(j == i),
                        )
                rc = mpool.tile([P, 2], F32, tag="rc", name=f"rc_{p}_{i}")
                nc.vector.reciprocal(rc[:], po[:, :, 64])
                for hh in (0, 1):
                    nc.vector.tensor_scalar_mul(
                        opair[:, 64 * hh : 64 * hh + 64], po[:, hh, 0:64], rc[:, hh : hh + 1]
                    )
                nc.sync.dma_start(out_d[i * P : (i + 1) * P, p * P : (p + 1) * P], opair[:])

            # Prologue: just the first QK stripes so S^T (0,0) can start ASAP.
            emit_qk(0, 0)
            emit_qk(4, 0)

            pv_queue = []
            blocks = [(pos, t, p) for pos, t in enumerate((0, 1, 2, 3)) for p in range(4)]
            for n, (pos, t, p) in enumerate(blocks):
                    for tt in range(t + 1):
                        emit_qk(p, tt)
                        emit_qk(4 + p, tt)
                    # QK tiles the NEXT block's S^T will need: emitted from this
                    # block's group hooks so the boundary has no QK burst.
                    nxt_qk = []
                    if n + 1 < len(blocks):
                        _, tn, pn = blocks[n + 1]
                        nxt_qk = [
                            (c, tt)
                            for tt in range(tn + 1)
                            for c in (pn, 4 + pn)
                            if (c, tt) not in done_qk
                        ]
                    # pt layout: [128, hh, chunk, 512]
                    pt = ptpool.tile([P, 2, 16, 512], F16, tag="pt", name=f"pt_{p}_{t}")
                    # V tiles this stripe's PV needs, spread across the group loop
                    vpend = [
                        (j, half)
                        for j in range(4 * t, 4 * t + 4)
                        for half in (0, 1)
                        if (j, half) not in done_v
                    ]

                    def group_hooks(pos=pos, vpend=vpend, nxt_qk=nxt_qk):
                        if pv_queue:
                            emit_pv(*pv_queue.pop(0))
                        if vpend:
                            emit_v(*vpend.pop(0))
                        if nxt_qk:
                            emit_qk(*nxt_qk.pop(0))
                            return
                        # pace filler: defer to the exp-heavy late rounds (prefix
                        # preconditions are met by the forced drains)
                        state["g"] = state.get("g", 0) + 1
                        if pos == 3:
                            pull(1)

                    # S^T + exp in groups of 2 chunks per head; diagonal chunks only
                    # compute the causal-valid columns (stale psum prefix is bounded
                    # old scores: exp'd then never consumed).
                    for g in range(2 * t + 2):
                        psA = ps_s.tile([P, 2, 512], F32, tag="s", name=f"psA_{p}_{t}_{g}")
                        psB = ps_s.tile([P, 2, 512], F32, tag="s", name=f"psB_{p}_{t}_{g}")
                        for jj in (0, 1):
                            j = 2 * g + jj
                            q0 = 128 * (j - 4 * t) if j >= 4 * t else 0
                            for hh, ps in ((0, psA), (1, psB)):
                                nc.tensor.matmul(
                                    ps[:, jj, q0:512],
                                    lhsT=kt_sb[:, 2 * p + hh, j * P : (j + 1) * P],
                                    rhs=qt_sb[:, p, t * 512 + q0 : (t + 1) * 512],
                                    start=True,
                                    stop=True,
                                )
                        for hh, ps in ((0, psA), (1, psB)):
                            if g == 2 * t + 1:
                                # fully-diagonal group: exp only the causal-valid
                                # suffixes (contiguous slices)
                                nc.scalar.activation(
                                    pt[:, hh, 2 * g, 256:512],
                                    ps[:, 0, 256:512],
                                    mybir.ActivationFunctionType.Exp,
                                    scale=0.125,
                                )
                                nc.scalar.activation(
                                    pt[:, hh, 2 * g + 1, 384:512],
                                    ps[:, 1, 384:512],
                                    mybir.ActivationFunctionType.Exp,
                                    scale=0.125,
                                )
                            else:
                                nc.scalar.activation(
                                    pt[:, hh, 2 * g : 2 * g + 2, :],
                                    ps[:],
                                    mybir.ActivationFunctionType.Exp,
                                    scale=0.125,
                                )
                        group_hooks()
                    while pv_queue:
                        emit_pv(*pv_queue.pop(0))
                    # causal mask on diagonal 128x128 blocks (GpSimd: it's idle, and
                    # this keeps DVE free for psum drains/epilogues)
                    for hh in (0, 1):
                        for r in range(4):
                            j = 4 * t + r
                            blk = pt[:, hh, j, r * P : (r + 1) * P]
                            nc.gpsimd.tensor_mul(blk, blk, tri_sb[:])
                    # V tiles this stripe's PV will need (PV runs during next stripe)
                    for j in range(4 * t + 4):
                        emit_v(j, 0)
                        emit_v(j, 1)
                    pv_queue = [(p, t, pt, r) for r in range(4)]
            while pv_queue:
                emit_pv(*pv_queue.pop(0))
            pull(len(filler))  # safety: flush

    nc.compile()
    return nc


def get_nc():
    if "nc" not in _cache:
        _cache["nc"] = _build()
    return _cache["nc"]


def _prep_core_inputs(x, W, b, bi, hg):
    h0 = hg * HL
    Wq = W[:, 0:D].reshape(D, H, HD)
    Wk = W[:, D : 2 * D].reshape(D, H, HD)
    Wv = W[:, 2 * D :].reshape(D, H, HD)
    bq = b[0:D].reshape(H, HD)
    bk = b[D : 2 * D].reshape(H, HD)
    bv = b[2 * D :].reshape(H, HD)

    # pair-major: pair p occupies cols [256p, 256p+256) as [Q pair | K pair]
    wqk = np.empty((D, 1024), np.float32)
    bqk = np.empty((P, 8), np.float32)
    for c in range(4):
        for half in range(2):
            h = h0 + 2 * c + half
            sl = slice(256 * c + half * HD, 256 * c + half * HD + HD)
            wqk[:, sl] = Wq[:, h]
            bqk[half * HD : (half + 1) * HD, c] = bq[h]
            sl = slice(256 * c + P + half * HD, 256 * c + P + half * HD + HD)
            wqk[:, sl] = Wk[:, h]
            bqk[half * HD : (half + 1) * HD, 4 + c] = bk[h]

    wv_aug = np.zeros((D, VW), np.float32)
    bv_aug = np.zeros((VW,), np.float32)
    for hl in range(HL):
        wv_aug[:, 65 * hl : 65 * hl + HD] = Wv[:, h0 + hl]
        bv_aug[65 * hl : 65 * hl + HD] = bv[h0 + hl]
        bv_aug[65 * hl + HD] = 1.0

    tri = np.triu(np.ones((P, P), np.float32))  # tri[k, q] = 1 where q >= k

    return {
        "x": np.ascontiguousarray(x[bi].astype(np.float16).T),
        "wqk": wqk.astype(np.float16),
        "wv": wv_aug.astype(np.float16),
        "bqk": bqk,
        "bv": bv_aug[None, :].astype(np.float16),
        "tri": tri.astype(np.float16),
        "ones1": np.ones((1, P), np.float16),
    }


def make_in_maps(x, W_qkv, b_qkv):
    x = np.asarray(x, dtype=np.float32)
    W = np.asarray(W_qkv, dtype=np.float32)
    b = np.asarray(b_qkv, dtype=np.float32)
    return [_prep_core_inputs(x, W, b, i // 2, i % 2) for i in range(N_CORES)]


def assemble(results):
    out = np.empty((B, N, D), np.float32)
    for i in range(N_CORES):
        bi, hg = i // 2, i % 2
        out[bi, :, hg * 512 : (hg + 1) * 512] = results[i]["out"]
    return out


def run(x, W_qkv, b_qkv, trace=False, tmpdir=None):
    nc = get_nc()
    in_maps = make_in_maps(x, W_qkv, b_qkv)
    res = bass_utils.run_bass_kernel_spmd(
        nc, in_maps, core_ids=list(range(N_CORES)), trace=trace, tmpdir=tmpdir
    )
    return assemble(res.results), res


def kernel(x, W_qkv, b_qkv):
    out, _ = run(x, W_qkv, b_qkv)
    return out


# revision 47
# speedup vs baseline: 1.0729x; 1.0005x over previous
"""Causal multi-head attention (QKV projection + softmax(QK^T)V) on 8 TRN2 NeuronCores.

Problem: x[4,2048,1024] @ W_qkv[1024,3072] + b_qkv -> 16-head causal attention -> [4,2048,1024].

Sharding: core i = (batch bi=i//2, head-group hg=i%2). Each core handles 1 batch x 8 heads,
fully data/tensor-parallel (no collectives). Host pre-arranges per-core inputs (all matmul
operands fp16; accumulation f32 in PSUM):
  - x passed pre-transposed [1024, 2048] so the contraction dim lands on partitions with
    plain contiguous DMAs (no on-device transposes anywhere).
  - wqk [1024,1024] pair-major (pair p: Q cols at 256p, K at 256p+128), head-PAIR-stacked
    (64+64 rows) so QKV^T matmul output chunks are directly the [hd, n] stacked layout the
    attention stage consumes, and the first S^T only waits on a 0.5 MB weight slice.
  - wv [1024,520]: V columns with per-head stride 65; col 65h+64 is a zero column whose
    bias entry is 1.0, so the "ones column" that makes the PV matmul accumulate softmax
    denominators is produced by the same K=1 bias-row matmul that applies b_v.
Device pipeline per core:
  QKV^T matmuls (Q^T pair-stacked, K^T zero-padded per head so S^T runs K=128 with fast
  weight load, V natural) -> S^T = K Q^T per key-chunk with causal column trimming ->
  one ScalarE Exp(scale=1/8) per 2-chunk group, PSUM->SBUF fp16 = P^T -> causal tri-mask
  multiply on the 128x128 diagonal blocks only (GpSimd) -> PV matmuls accumulate
  [q, 64 cols + denominator] per q-block (both heads packed in one PSUM bank) ->
  reciprocal * scale epilogue (DVE) -> DMA out [2048, 512] f32.
Scheduling: ScalarE exp (~155us) and TensorE (~204us) must overlap near-perfectly.
Attention runs stripe-major across head-pairs (t-major rounds); QKV matmul tiles are a
deadline-ordered "filler" queue drained between S^T groups (next block's QK tiles are
prefetched from the current block's hooks); each stripe's PV is deferred into the next
stripe's S^T/exp loop. All bulk DMA rides the sync HWDGE ring so ScalarE's instruction
stream is pure exps.
"""

import numpy as np

import concourse.bass as bass
import concourse.tile as tile
from concourse import bacc, mybir
from concourse import bass_utils

F16 = mybir.dt.float16
F32 = mybir.dt.float32

B, N, D = 4, 2048, 1024
H = 16  # global heads
HD = 64
HL = 8  # heads per core
N_CORES = 8
P = 128
NT = N // P  # 16 token tiles
KC = D // P  # 8 contraction chunks
VW = HL * (HD + 1)  # 520
VH = VW // 2  # 260

_cache = {}


def _build():
    nc = bacc.Bacc("TRN2", target_bir_lowering=False, debug=False)

    x_d = nc.dram_tensor("x", [D, N], F16, kind="ExternalInput").ap()  # x^T, host-transposed
    wqk_d = nc.dram_tensor("wqk", [D, 1024], F16, kind="ExternalInput").ap()
    wv_d = nc.dram_tensor("wv", [D, VW], F16, kind="ExternalInput").ap()
    bqk_d = nc.dram_tensor("bqk", [P, 8], F32, kind="ExternalInput").ap()
    bv_d = nc.dram_tensor("bv", [1, VW], F16, kind="ExternalInput").ap()
    tri_d = nc.dram_tensor("tri", [P, P], F16, kind="ExternalInput").ap()
    ones_d = nc.dram_tensor("ones1", [1, P], F16, kind="ExternalInput").ap()
    out_d = nc.dram_tensor("out", [N, HL * HD], F32, kind="ExternalOutput").ap()

    wqk_r = wqk_d.rearrange("(k p) n -> p k n", p=P)
    wv_r = wv_d.rearrange("(k p) n -> p k n", p=P)

    with tile.TileContext(nc) as tc:
        with (
            tc.tile_pool(name="const", bufs=1) as cpool,
            tc.tile_pool(name="pt", bufs=2) as ptpool,
            tc.tile_pool(name="opair", bufs=3) as oppool,
            tc.tile_pool(name="misc", bufs=6) as mpool,
            tc.tile_pool(name="ps_mm", bufs=2, space="PSUM") as ps_mm,
            tc.tile_pool(name="ps_s", bufs=2, space="PSUM") as ps_s,
            tc.tile_pool(name="ps_o", bufs=2, space="PSUM") as ps_o,
        ):
            # ---- constants / inputs to SBUF ----
            xt_sb = cpool.tile([P, KC, N], F16, name="xt_sb")  # x^T, 8 chunks of [128, 2048]
            wqk_sb = cpool.tile([P, KC, 1024], F16, name="wqk_sb")
            wv_sb = cpool.tile([P, KC, VW], F16, name="wv_sb")
            bqk_sb = cpool.tile([P, 8], F32, name="bqk_sb")
            bv_sb = cpool.tile([1, VW], F16, name="bv_sb")
            tri_sb = cpool.tile([P, P], F16, name="tri_sb")
            ones_sb = cpool.tile([1, P], F16, name="ones_sb")
            qt_sb = cpool.tile([P, 4, N], F16, name="qt_sb")  # Q^T pair-stacked
            # K^T zero-padded per head: head h_l occupies rows 64*(h_l%2).. , other half 0,
            # so S^T matmuls run K=128 (FWL) against the pair-stacked Q^T rhs.
            kt_sb = cpool.tile([P, HL, N], F16, name="kt_sb")
            v_sb = cpool.tile([P, NT, VW], F16, name="v_sb")

            nc.gpsimd.memset(kt_sb[:], 0.0)
            # Preload the exp table set (~2.7us) during the DMA fill, so the first
            # real softmax exp doesn't pay ACT_TABLE_LOAD.
            warm = mpool.tile([1, 8], F32, tag="warm", name="warm")
            nc.gpsimd.memset(warm[:], 0.0)
            nc.scalar.activation(warm[:], warm[:], mybir.ActivationFunctionType.Exp)
            # ALL bulk input DMA rides the sync HWDGE ring: the scalar ring would put
            # DMA-trigger instructions (which stall on ring credit) ahead of the first
            # EXPs in ScalarE's instruction stream, delaying softmax by ~20us.
            nc.sync.dma_start(bqk_sb[:], bqk_d)
            nc.sync.dma_start(tri_sb[:], tri_d)
            nc.sync.dma_start(bv_sb[:], bv_d)
            nc.sync.dma_start(ones_sb[:], ones_d)

            # x^T streamed stripe-major in [128,512] pieces: stripe-0 of all chunks
            # lands first so QK(.,0) -> S^T(0,0) -> exp starts ~10us in.
            def x_piece(k, tt):
                nc.sync.dma_start(
                    xt_sb[:, k, tt * 512 : (tt + 1) * 512],
                    x_d[k * P : (k + 1) * P, tt * 512 : (tt + 1) * 512],
                )

            # wqk is pair-major host-side (pair p: Q at 256p, K at 256p+128), so the
            # first S^T only waits on the 256-col pair-0 pieces.
            for k in range(KC):
                x_piece(k, 0)
                nc.sync.dma_start(wqk_sb[:, k, 0:256], wqk_r[:, k, 0:256])
            for k in range(KC):
                nc.sync.dma_start(wv_sb[:, k, :], wv_r[:, k, :])
            for k in range(KC):
                nc.sync.dma_start(wqk_sb[:, k, 256:1024], wqk_r[:, k, 256:1024])
            for k in range(KC):
                x_piece(k, 1)
            for k in range(KC):
                x_piece(k, 2)
            for k in range(KC):
                x_piece(k, 3)

            done_qk = set()
            done_v = set()

            def emit_qk(c, tt):
                """QKV^T matmul tile for col-chunk c, token stripe tt."""
                if (c, tt) in done_qk:
                    return
                done_qk.add((c, tt))
                pr = c % 4
                pq = ps_mm.tile([P, 512], F32, tag="mm", name=f"pq_{c}_{tt}")
                col0 = 256 * (c % 4) + (0 if c < 4 else 128)
                for k in range(KC):
                    nc.tensor.matmul(
                        pq[:],
                        lhsT=wqk_sb[:, k, col0 : col0 + P],
                        rhs=xt_sb[:, k, tt * 512 : (tt + 1) * 512],
                        start=(k == 0),
                        stop=(k == KC - 1),
                    )
                def badd(out, in_, b):
                    nc.vector.tensor_scalar_add(out, in_, b)

                if c < 4:
                    badd(
                        qt_sb[:, pr, tt * 512 : (tt + 1) * 512], pq[:], bqk_sb[:, c : c + 1]
                    )
                else:
                    for hh in (0, 1):
                        rows = slice(64 * hh, 64 * hh + 64)
                        badd(
                            kt_sb[rows, 2 * pr + hh, tt * 512 : (tt + 1) * 512],
                            pq[rows, :],
                            bqk_sb[rows, c : c + 1],
                        )

            def emit_v(j, half):
                """V (augmented) for token tile j, half (260 cols each)."""
                if (j, half) in done_v:
                    return
                done_v.add((j, half))
                pv = ps_mm.tile([P, VH], F32, tag="mm", name=f"pv_{j}_{half}")
                for k in range(KC):
                    nc.tensor.matmul(
                        pv[:],
                        lhsT=xt_sb[:, k, j * P : (j + 1) * P],
                        rhs=wv_sb[:, k, half * VH : (half + 1) * VH],
                        start=(k == 0),
                        stop=False,
                    )
                nc.tensor.matmul(
                    pv[:],
                    lhsT=ones_sb[0:1, :],
                    rhs=bv_sb[0:1, half * VH : (half + 1) * VH],
                    start=False,
                    stop=True,
                )
                nc.vector.tensor_copy(v_sb[:, j, half * VH : (half + 1) * VH], pv[:])

            # Filler queue: PE work pulled into the attention loop between S^T groups,
            # ordered to match t-major demand (QK stripes tt, then V tiles of stripe tt).
            filler = []
            for tt in range(4):
                for pr in range(4):
                    if (pr, tt) != (0, 0):
                        filler += [("qk", pr, tt), ("qk", pr + 4, tt)]
                filler += [("v", j, half) for j in range(4 * tt, 4 * tt + 4) for half in (0, 1)]
            state = {"i": 0}

            def pull(n):
                while n > 0 and state["i"] < len(filler):
                    it = filler[state["i"]]
                    state["i"] += 1
                    if it[0] == "v":
                        if (it[1], it[2]) in done_v:
                            continue
                        emit_v(it[1], it[2])
                    else:
                        if (it[1], it[2]) in done_qk:
                            continue
                        emit_qk(it[1], it[2])
                    n -= 1

            def emit_pv(p, t, pt, r):
                """PV + epilogue + out DMA for q-block i = 4t+r of pair p.
                Both heads' accumulators share one PSUM bank ([128, 2, 65])."""
                i = 4 * t + r
                opair = oppool.tile([P, P], F32, tag="op", name=f"op_{p}_{i}")
                po = ps_o.tile([P, 2, 65], F32, tag="o", name=f"po_{p}_{i}")
                for hh in (0, 1):
                    for j in range(i + 1):
                        nc.tensor.matmul(
                            po[:, hh, :],
                            lhsT=pt[:, hh, j, r * P : (r + 1) * P],
                            rhs=v_sb[:, j, 65 * (2 * p + hh) : 65 * (2 * p + hh) + 65],
                            start=(j == 0),
                            stop=(j == i),
                        )
                rc = mpool.tile([P, 2], F32, tag="rc", name=f"rc_{p}_{i}")
                nc.vector.reciprocal(rc[:], po[:, :, 64])
                for hh in (0, 1):
                    nc.vector.tensor_scalar_mul(
                        opair[:, 64 * hh : 64 * hh + 64], po[:, hh, 0:64], rc[:, hh : hh + 1]
                    )
                nc.sync.dma_start(out_d[i * P : (i + 1) * P, p * P : (p + 1) * P], opair[:])

            # Prologue: just the first QK stripes so S^T (0,0) can start ASAP.
            emit_qk(0, 0)
            emit_qk(4, 0)

            pv_queue = []
            blocks = [(pos, t, p) for pos, t in enumerate((0, 1, 2, 3)) for p in range(4)]
            for n, (pos, t, p) in enumerate(blocks):
                    for tt in range(t + 1):
                        emit_qk(p, tt)
                        emit_qk(4 + p, tt)
                    # QK tiles the NEXT block's S^T will need: emitted from this
                    # block's group hooks so the boundary has no QK burst.
                    nxt_qk = []
                    if n + 1 < len(blocks):
                        _, tn, pn = blocks[n + 1]
                        nxt_qk = [
                            (c, tt)
                            for tt in range(tn + 1)
                            for c in (pn, 4 + pn)
                            if (c, tt) not in done_qk
                        ]
                    # pt layout: [128, hh, chunk, 512]
                    pt = ptpool.tile([P, 2, 16, 512], F16, tag="pt", name=f"pt_{p}_{t}")
                    # V tiles this stripe's PV needs, spread across the group loop
                    vpend = [
                        (j, half)
                        for j in range(4 * t, 4 * t + 4)
                        for half in (0, 1)
                        if (j, half) not in done_v
                    ]

                    def group_hooks(pos=pos, vpend=vpend, nxt_qk=nxt_qk):
                        if pv_queue:
                            emit_pv(*pv_queue.pop(0))
                        if vpend:
                            emit_v(*vpend.pop(0))
                        if nxt_qk:
                            emit_qk(*nxt_qk.pop(0))
                            return
                        # pace filler: defer to the exp-heavy late rounds (prefix
                        # preconditions are met by the forced drains)
                        state["g"] = state.get("g", 0) + 1
                        if pos == 3:
                            pull(1)

                    # S^T + exp in groups of 2 chunks per head; diagonal chunks only
                    # compute the causal-valid columns (stale psum prefix is bounded
                    # old scores: exp'd then never consumed).
                    for g in range(2 * t + 2):
                        psA = ps_s.tile([P, 2, 512], F32, tag="s", name=f"psA_{p}_{t}_{g}")
                        psB = ps_s.tile([P, 2, 512], F32, tag="s", name=f"psB_{p}_{t}_{g}")
                        for jj in (0, 1):
                            j = 2 * g + jj
                            q0 = 128 * (j - 4 * t) if j >= 4 * t else 0
                            for hh, ps in ((0, psA), (1, psB)):
                                nc.tensor.matmul(
                                    ps[:, jj, q0:512],
                                    lhsT=kt_sb[:, 2 * p + hh, j * P : (j + 1) * P],
                                    rhs=qt_sb[:, p, t * 512 + q0 : (t + 1) * 512],
                                    start=True,
                                    stop=True,
                                )
                        for hh, ps in ((0, psA), (1, psB)):
                            if g == 2 * t + 1:
                                # fully-diagonal group: exp only the causal-valid
                                # suffixes (contiguous slices)
                                nc.scalar.activation(
                                    pt[:, hh, 2 * g, 256:512],
                                    ps[:, 0, 256:512],
                                    mybir.ActivationFunctionType.Exp,
                                    scale=0.125,
                                )
                                nc.scalar.activation(
                                    pt[:, hh, 2 * g + 1, 384:512],
                                    ps[:, 1, 384:512],
                                    mybir.ActivationFunctionType.Exp,
                                    scale=0.125,
                                )
                            else:
                                nc.scalar.activation(
                                    pt[:, hh, 2 * g : 2 * g + 2, :],
                                    ps[:],
                                    mybir.ActivationFunctionType.Exp,
                                    scale=0.125,
                                )
                        group_hooks()
                    while pv_queue:
                        emit_pv(*pv_queue.pop(0))
                    # causal mask on diagonal 128x128 blocks (GpSimd: it's idle, and
                    # this keeps DVE free for psum drains/epilogues)
                    for hh in (0, 1):
                        for r in range(4):
                            j = 4 * t + r
                            blk = pt[:, hh, j, r * P : (r + 1) * P]
                            nc.gpsimd.tensor_mul(blk, blk, tri_sb[:])
                    # V tiles this stripe's PV will need (PV runs during next stripe)
                    for j in range(4 * t + 4):
                        emit_v(j, 0)
                        emit_v(j, 1)
                    pv_queue = [(p, t, pt, r) for r in range(4)]
            while pv_queue:
                emit_pv(*pv_queue.pop(0))
            pull(len(filler))  # safety: flush

    nc.compile()
    return nc


def get_nc():
    if "nc" not in _cache:
        _cache["nc"] = _build()
    return _cache["nc"]


def _prep_core_inputs(x, W, b, bi, hg):
    h0 = hg * HL
    Wq = W[:, 0:D].reshape(D, H, HD)
    Wk = W[:, D : 2 * D].reshape(D, H, HD)
    Wv = W[:, 2 * D :].reshape(D, H, HD)
    bq = b[0:D].reshape(H, HD)
    bk = b[D : 2 * D].reshape(H, HD)
    bv = b[2 * D :].reshape(H, HD)

    # pair-major: pair p occupies cols [256p, 256p+256) as [Q pair | K pair]
    wqk = np.empty((D, 1024), np.float32)
    bqk = np.empty((P, 8), np.float32)
    for c in range(4):
        for half in range(2):
            h = h0 + 2 * c + half
            sl = slice(256 * c + half * HD, 256 * c + half * HD + HD)
            wqk[:, sl] = Wq[:, h]
            bqk[half * HD : (half + 1) * HD, c] = bq[h]
            sl = slice(256 * c + P + half * HD, 256 * c + P + half * HD + HD)
            wqk[:, sl] = Wk[:, h]
            bqk[half * HD : (half + 1) * HD, 4 + c] = bk[h]

    wv_aug = np.zeros((D, VW), np.float32)
    bv_aug = np.zeros((VW,), np.float32)
    for hl in range(HL):
        wv_aug[:, 65 * hl : 65 * hl + HD] = Wv[:, h0 + hl]
        bv_aug[65 * hl : 65 * hl + HD] = bv[h0 + hl]
        bv_aug[65 * hl + HD] = 1.0

    tri = np.triu(np.ones((P, P), np.float32))  # tri[k, q] = 1 where q >= k

    return {
        "x": np.ascontiguousarray(x[bi].astype(np.float16).T),
        "wqk": wqk.astype(np.float16),
        "wv": wv_aug.astype(np.float16),
        "bqk": bqk,
        "bv": bv_aug[None, :].astype(np.float16),
        "tri": tri.astype(np.float16),
        "ones1": np.ones((1, P), np.float16),
    }


def make_in_maps(x, W_qkv, b_qkv):
    x = np.asarray(x, dtype=np.float32)
    W = np.asarray(W_qkv, dtype=np.float32)
    b = np.asarray(b_qkv, dtype=np.float32)
    return [_prep_core_inputs(x, W, b, i // 2, i % 2) for i in range(N_CORES)]


def assemble(results):
    out = np.empty((B, N, D), np.float32)
    for i in range(N_CORES):
        bi, hg = i // 2, i % 2
        out[bi, :, hg * 512 : (hg + 1) * 512] = results[i]["out"]
    return out


def run(x, W_qkv, b_qkv, trace=False, tmpdir=None):
    nc = get_nc()
    in_maps = make_in_maps(x, W_qkv, b_qkv)
    res = bass_utils.run_bass_kernel_spmd(
        nc, in_maps, core_ids=list(range(N_CORES)), trace=trace, tmpdir=tmpdir
    )
    return assemble(res.results), res


def kernel(x, W_qkv, b_qkv):
    out, _ = run(x, W_qkv, b_qkv)
    return out
